# revision 6
# baseline (speedup 1.0000x reference)
"""EnergyBasedVAD Trainium2 kernel.

Input:  waveform (32, 960000) f32.
Output: (32, 3749) bool VAD mask.

Sharding: pure data parallel — 4 batch rows per core across 8 cores.

Device computes short-time energy (the memory-bound part: 123 MB of
waveform reads). Each row of 960000 samples is 125 partitions x 7680
samples (30 blocks of 256, no halo). The row is loaded in 3 column
slices of 10 blocks (1.28 MB each) so compute starts ~3us after the
first slice lands instead of waiting ~21us for a whole-row transfer;
all input slices ride the sync-engine HWDGE ring (FIFO order => the
head transfer gets the full 16-SDMA bandwidth), outputs ride the
scalar ring so they never head-of-line-block the input stream.

Each slice is squared on ACT (mean's 1/512 folded into the activation
scale), block-summed 64 -> 256 on DVE into a per-row [125, 30] tile of
block energies, which is DMA'd out raw. The final pair-add
(frame t = block t + block t+1, which crosses partition boundaries
every 30th frame) runs on host over the (32, 3750) block sums — same
f32 adds, bit-identical, and it saves three DVE ops per row plus any
halo re-reads.

Host also computes the 20%-quantile threshold and the hysteresis
segment state machine on the (32, 3749) energies — 0.01% of the bytes.
"""

import math
import numpy as np

import concourse.bass as bass
import concourse.bacc as bacc
import concourse.mybir as mybir
from concourse.bass_utils import run_bass_kernel_spmd
from concourse.tile import TileContext

N_CORES = 8
B, S = 32, 960000
ROWS = B // N_CORES          # 4 rows per core
PV = 125                     # partitions per row
SEG = 7680                   # samples per partition = 30 blocks of 256
NBLK_P = 30                  # 256-blocks per partition
NBLK = S // 256              # 3750 block sums per row
T = (S - 512) // 256 + 1     # 3749 output frames
FLAT = ROWS * S

K_SLICES = 3
BLK_SL = 10                  # blocks per slice
CSL = BLK_SL * 256           # 2560 columns per slice

SILENCE_FRAMES = 18
MIN_SPEECH_FRAMES = 6
ENERGY_THRESHOLD = 0.01

_CACHE = {}


def _build(repeat: int = 1):
    nc = bacc.Bacc(None)
    wav = nc.declare_dram_parameter("waveform", [FLAT], mybir.dt.float32, isOutput=False)
    eout = nc.declare_dram_parameter("energy", [ROWS, NBLK], mybir.dt.float32, isOutput=True)

    inv = 1.0 / math.sqrt(512.0)
    sq_t = mybir.ActivationFunctionType.Square

    with TileContext(nc) as tc:
        with (
            tc.tile_pool(name="wav", bufs=6) as wav_pool,
            tc.tile_pool(name="sq", bufs=3) as sq_pool,
            tc.tile_pool(name="c64", bufs=2) as c64_pool,
            tc.tile_pool(name="c256", bufs=2) as c256_pool,
        ):
            for i in range(ROWS * repeat):
                r = i % ROWS
                c64t = c64_pool.tile([128, K_SLICES * BLK_SL * 4], mybir.dt.float32)
                for k in range(K_SLICES):
                    wt = wav_pool.tile([128, CSL], mybir.dt.float32)
                    nc.sync.dma_start(
                        out=wt[0:PV, :],
                        in_=bass.AP(wav, r * S + k * CSL, [[SEG, PV], [1, CSL]]),
                    )
                    sq = sq_pool.tile([128, CSL], mybir.dt.float32)
                    nc.scalar.activation(sq[0:PV, :], wt[0:PV, :], sq_t, scale=inv)
                    nc.vector.reduce_sum(
                        c64t[0:PV, 4 * BLK_SL * k: 4 * BLK_SL * (k + 1)],
                        sq[0:PV, :].rearrange("p (n f) -> p n f", f=64),
                        axis=mybir.AxisListType.X,
                    )
                c256 = c256_pool.tile([128, NBLK_P], mybir.dt.float32)
                nc.vector.reduce_sum(
                    c256[0:PV, :],
                    c64t[0:PV, :].rearrange("p (n f) -> p n f", f=4),
                    axis=mybir.AxisListType.X,
                )
                nc.scalar.dma_start(
                    out=eout[r].rearrange("(p x) -> p x", p=PV), in_=c256[0:PV, :]
                )
    nc.finalize()
    return nc


def _in_maps(waveform: np.ndarray):
    w = np.ascontiguousarray(waveform, dtype=np.float32)
    return [
        {"waveform": w[c * ROWS:(c + 1) * ROWS].ravel()} for c in range(N_CORES)
    ]


def _run_device(waveform: np.ndarray, trace: bool = False, trace_cores=None):
    if "nc" not in _CACHE:
        _CACHE["nc"] = _build()
    nc = _CACHE["nc"]
    res = run_bass_kernel_spmd(
        nc, _in_maps(waveform), core_ids=list(range(N_CORES)), trace=trace,
        trace_cores=trace_cores,
    )
    blocks = np.concatenate([res.results[c]["energy"] for c in range(N_CORES)], axis=0)
    energy = blocks[:, :-1] + blocks[:, 1:]    # frame t = block t + block t+1
    return energy, res


def _vad_from_energy(e: np.ndarray) -> np.ndarray:
    """Threshold + hysteresis state machine, faithful to the reference."""
    n = e.shape[1]
    out = np.zeros((e.shape[0], n), dtype=bool)
    for b in range(e.shape[0]):
        s = np.sort(e[b])
        nzero = int((s <= 0).sum())
        nz = n - nzero
        if nz > 0:
            pos = np.float32(0.2) * np.float32(nz - 1)
            lo = int(np.floor(pos))
            hi = int(np.ceil(pos))
            frac = np.float32(pos) - np.float32(lo)
            ilo = min(max(nzero + lo, 0), n - 1)
            ihi = min(max(nzero + hi, 0), n - 1)
            thr = np.float32(s[ilo] * (np.float32(1.0) - frac) + s[ihi] * frac)
        else:
            thr = np.float32(ENERGY_THRESHOLD)
        m = e[b] > thr
        t = np.nonzero(m)[0]
        if len(t) == 0:
            continue
        grp = np.concatenate([[0], (np.diff(t) > SILENCE_FRAMES).cumsum()])
        for g in range(grp[-1] + 1):
            tg = t[grp == g]
            first, last = int(tg[0]), int(tg[-1])
            if last >= n - SILENCE_FRAMES:
                st, en = first, n      # trailing open segment
            else:
                st, en = first, last   # closed: end excludes last speech frame
            if en - st >= MIN_SPEECH_FRAMES:
                out[b, st:en] = True
    return out


def kernel(waveform: np.ndarray, _trace: bool = False) -> np.ndarray:
    energy, res = _run_device(waveform, trace=_trace)
    _CACHE["last_result"] = res
    return _vad_from_energy(energy)


# revision 9
# speedup vs baseline: 2.0989x; 2.0989x over previous
"""EnergyBasedVAD Trainium2 kernel.

Input:  waveform (32, 960000) f32.
Output: (32, 3749) bool VAD mask.

Sharding: pure data parallel — 4 batch rows per core across 8 cores.

Device computes short-time energy (the memory-bound part: 123 MB of
waveform reads). Each row of 960000 samples is 125 partitions x 7680
samples (30 blocks of 256, no halo). The row is loaded in 3 column
slices of 10 blocks (1.28 MB each) so compute starts ~3us after the
first slice lands instead of waiting ~21us for a whole-row transfer;
all input slices ride the sync-engine HWDGE ring (FIFO order => the
head transfer gets the full 16-SDMA bandwidth), outputs ride the
scalar ring so they never head-of-line-block the input stream.

Each slice is squared on ACT (mean's 1/512 folded into the activation
scale), block-summed 64 -> 256 on DVE into a per-row [125, 30] tile of
block energies, which is DMA'd out raw. The final pair-add
(frame t = block t + block t+1, which crosses partition boundaries
every 30th frame) runs on host over the (32, 3750) block sums — same
f32 adds, bit-identical, and it saves three DVE ops per row plus any
halo re-reads.

Host also computes the 20%-quantile threshold and the hysteresis
segment state machine on the (32, 3749) energies — 0.01% of the bytes.
"""

import math
import numpy as np

import concourse.bass as bass
import concourse.bacc as bacc
import concourse.mybir as mybir
from concourse.bass_utils import run_bass_kernel_spmd
from concourse.tile import TileContext

N_CORES = 8
B, S = 32, 960000
ROWS = B // N_CORES          # 4 rows per core
PV = 125                     # partitions per row
SEG = 7680                   # samples per partition = 30 blocks of 256
NBLK_P = 30                  # 256-blocks per partition
NBLK = S // 256              # 3750 block sums per row
T = (S - 512) // 256 + 1     # 3749 output frames
# DMA slices carry 128 partition-lines (the HWDGE splits a transfer's lines
# across SDMA engines as count/(largest divisor <= 16): 128 lines -> 16
# engines, 125 lines -> only 5). Partitions 125-127 read past the row into
# the pad (garbage, discarded); pad covers the last row's overhang.
PDMA = 128
FLAT = ROWS * S + (PDMA - PV) * SEG

K_SLICES = 3
BLK_SL = 10                  # blocks per slice
CSL = BLK_SL * 256           # 2560 columns per slice

SILENCE_FRAMES = 18
MIN_SPEECH_FRAMES = 6
ENERGY_THRESHOLD = 0.01

_CACHE = {}


def _build(repeat: int = 1):
    nc = bacc.Bacc(None)
    wav = nc.declare_dram_parameter("waveform", [FLAT], mybir.dt.float32, isOutput=False)
    eout = nc.declare_dram_parameter("energy", [ROWS, NBLK], mybir.dt.float32, isOutput=True)

    inv = 1.0 / math.sqrt(512.0)
    sq_t = mybir.ActivationFunctionType.Square

    with TileContext(nc) as tc:
        with (
            tc.tile_pool(name="wav", bufs=6) as wav_pool,
            tc.tile_pool(name="sq", bufs=3) as sq_pool,
            tc.tile_pool(name="c64", bufs=2) as c64_pool,
            tc.tile_pool(name="c256", bufs=2) as c256_pool,
        ):
            for i in range(ROWS * repeat):
                r = i % ROWS
                c64t = c64_pool.tile([128, K_SLICES * BLK_SL * 4], mybir.dt.float32)
                for k in range(K_SLICES):
                    wt = wav_pool.tile([128, CSL], mybir.dt.float32)
                    nc.sync.dma_start(
                        out=wt[:],
                        in_=bass.AP(wav, r * S + k * CSL, [[SEG, PDMA], [1, CSL]]),
                    )
                    sq = sq_pool.tile([128, CSL], mybir.dt.float32)
                    nc.scalar.activation(sq[0:PV, :], wt[0:PV, :], sq_t, scale=inv)
                    nc.vector.reduce_sum(
                        c64t[0:PV, 4 * BLK_SL * k: 4 * BLK_SL * (k + 1)],
                        sq[0:PV, :].rearrange("p (n f) -> p n f", f=64),
                        axis=mybir.AxisListType.X,
                    )
                c256 = c256_pool.tile([128, NBLK_P], mybir.dt.float32)
                nc.vector.reduce_sum(
                    c256[0:PV, :],
                    c64t[0:PV, :].rearrange("p (n f) -> p n f", f=4),
                    axis=mybir.AxisListType.X,
                )
                nc.scalar.dma_start(
                    out=eout[r].rearrange("(p x) -> p x", p=PV), in_=c256[0:PV, :]
                )
    nc.finalize()
    return nc


def _in_maps(waveform: np.ndarray):
    w = np.ascontiguousarray(waveform, dtype=np.float32)
    pad = np.zeros(FLAT - ROWS * S, np.float32)
    return [
        {"waveform": np.concatenate([w[c * ROWS:(c + 1) * ROWS].ravel(), pad])}
        for c in range(N_CORES)
    ]


def _run_device(waveform: np.ndarray, trace: bool = False, trace_cores=None):
    if "nc" not in _CACHE:
        _CACHE["nc"] = _build()
    nc = _CACHE["nc"]
    res = run_bass_kernel_spmd(
        nc, _in_maps(waveform), core_ids=list(range(N_CORES)), trace=trace,
        trace_cores=trace_cores,
    )
    blocks = np.concatenate([res.results[c]["energy"] for c in range(N_CORES)], axis=0)
    energy = blocks[:, :-1] + blocks[:, 1:]    # frame t = block t + block t+1
    return energy, res


def _vad_from_energy(e: np.ndarray) -> np.ndarray:
    """Threshold + hysteresis state machine, faithful to the reference."""
    n = e.shape[1]
    out = np.zeros((e.shape[0], n), dtype=bool)
    for b in range(e.shape[0]):
        s = np.sort(e[b])
        nzero = int((s <= 0).sum())
        nz = n - nzero
        if nz > 0:
            pos = np.float32(0.2) * np.float32(nz - 1)
            lo = int(np.floor(pos))
            hi = int(np.ceil(pos))
            frac = np.float32(pos) - np.float32(lo)
            ilo = min(max(nzero + lo, 0), n - 1)
            ihi = min(max(nzero + hi, 0), n - 1)
            thr = np.float32(s[ilo] * (np.float32(1.0) - frac) + s[ihi] * frac)
        else:
            thr = np.float32(ENERGY_THRESHOLD)
        m = e[b] > thr
        t = np.nonzero(m)[0]
        if len(t) == 0:
            continue
        grp = np.concatenate([[0], (np.diff(t) > SILENCE_FRAMES).cumsum()])
        for g in range(grp[-1] + 1):
            tg = t[grp == g]
            first, last = int(tg[0]), int(tg[-1])
            if last >= n - SILENCE_FRAMES:
                st, en = first, n      # trailing open segment
            else:
                st, en = first, last   # closed: end excludes last speech frame
            if en - st >= MIN_SPEECH_FRAMES:
                out[b, st:en] = True
    return out


def kernel(waveform: np.ndarray, _trace: bool = False) -> np.ndarray:
    energy, res = _run_device(waveform, trace=_trace)
    _CACHE["last_result"] = res
    return _vad_from_energy(energy)
